# revision 33
# baseline (speedup 1.0000x reference)
"""Trainium2 Bass kernel for softmax-free attention:
    q = x @ Wq^T; k = x @ Wk^T; v = x @ Wv^T
    s = (q @ k^T) / sqrt(d); out = s @ v
  x: [4, 4096, 1024], W*: [1024, 1024], out: [4, 4096, 1024] (fp32)

There is no softmax, so the attention is linear and reassociates:
    out = x . (Wq^T Wk) . (x^T x) . Wv^T / sqrt(d)
which replaces the two [L,L]-score matmuls (2*L^2*D flops/batch) with a
[D,D] Gram-matrix chain (2*L*D^2 + small), a ~4.3x flop reduction.

Sharding: 8 cores; core c handles batch c//2, row-half c%2 (2048 rows).
Per core:
  G0    = xo^T xo                  (own-row Gram partial, [1024,1024])
  CT    = Wk^T wqh                 (wqh = Wq[:, own 512 d-cols]/sqrt(d))
  AT    = (G0 + G1) CT             (G1 = partner Gram via shared DRAM)
  Hown  = AT^T WvT                 ([512,1024] own-d rows of H)
  out   = xoT^T [Hown; Hpeer]      (partner H half via shared DRAM)
The pair exchanges G0 (4MB) and H halves (2MB) through cross-core-visible
Shared-DRAM slots; ordering is a token AllReduce whose token is DMA-sampled
from the shared buffers (RAW dep on the spill writes). The AT accumulation
runs G0's 8 chunks before the barrier and G1's 8 chunks after, hiding the
barrier latency; the out accumulation orders own-H chunks first for the same
reason. Host-side, xoT rows are rotated own-d-half-first so every tile index
is static across the SPMD program (only the shared-DRAM slot is dynamic).

All matmul inputs are float32r (full PE rate at free-dim>=256); 1/sqrt(d)
is folded into wqh on the host.
"""

import sys
import types
from contextlib import ExitStack

import numpy as np

import concourse.bass as bass
import concourse.tile as tile
from concourse import bacc, mybir
from concourse.bass_utils import run_bass_kernel_spmd
from concourse.mybir import EngineType
from concourse.tile import add_dep_helper
from concourse.vector_clock import ScopedClock

# ---------------------------------------------------------------------------
# Environment shims
# ---------------------------------------------------------------------------


def _install_tile_drain_patch():
    """This toolchain's walrus caps sync waits at 1 per instruction, but
    TileContext's tail drain can carry several. Split the overflow onto
    preceding nops (same semantics: the issuing engine observes every sem
    before draining)."""
    if getattr(tile.TileContext, "_drain_patch_installed", False):
        return

    def _patched_drain_and_barrier(self, tick_clock, wait_clock):
        nc = self.nc
        collector = nc.sync.nop(hint="drain_wait_collector", nofuse=True)
        wait_clock.add_sem_waits(
            collector.ins, ScopedClock({None: tick_clock.global_clock})
        )
        waits = list(collector.ins.sync_info.on_wait or [])
        if len(waits) > 1:
            collector.ins.sync_info.on_wait = [waits[0]]
            for w in waits[1:]:
                nop = nc.sync.nop(hint="drain_wait_extra", nofuse=True)
                nop.ins.sync_info = mybir.SyncInfo(on_wait=[w], on_update=[])
        nc.sync.drain()

        nc.all_engine_barrier()
        assert self.sems is not None
        popped = nc._tile_sem_poison_stack.pop()
        assert popped is self._sem_poison
        nc.clear_and_free_semaphores(list(self.sems.allocated().values()))
        nc.all_engine_barrier()

    tile.TileContext._drain_and_barrier = _patched_drain_and_barrier
    tile.TileContext._drain_patch_installed = True


def _install_ntff_shim():
    """The image's antenv lacks axon_hooks, which silently degrades
    trace=True. Recreate the get/set pair and register the ctypes NTFF hook
    from trn_agent_boot (no-op if unavailable)."""
    if "antenv.axon_hooks" in sys.modules:
        return
    state = {"hook": None}

    def set_axon_ntff_profile_hook(h):
        state["hook"] = h

    def get_axon_ntff_profile_hook():
        return state["hook"]

    mod = types.ModuleType("antenv.axon_hooks")
    mod.set_axon_ntff_profile_hook = set_axon_ntff_profile_hook
    mod.get_axon_ntff_profile_hook = get_axon_ntff_profile_hook
    sys.modules["antenv.axon_hooks"] = mod
    try:
        import antenv

        antenv.axon_hooks = mod
        from trn_agent_boot.trn_boot import _ntff_profile_via_ctypes

        set_axon_ntff_profile_hook(
            _ntff_profile_via_ctypes("/opt/axon/libaxon_pjrt.so")
        )
    except Exception:
        pass


_install_tile_drain_patch()
_install_ntff_shim()

# ---------------------------------------------------------------------------
# Problem constants (hardcoded per the harness contract)
# ---------------------------------------------------------------------------

B, L, D = 4, 4096, 1024
N_CORES = 8
P = 128
LH = L // 2       # rows per core
DC = D // P       # 8 chunks of 128 over d/e/t/u
XC = LH // P      # 16 row-chunks of own x
DH = D // 2       # 512: own d-half width
F32 = mybir.dt.float32
F32R = mybir.dt.float32r
FH = 512          # matmul free-dim / psum group width

PAIRS = [[2 * i, 2 * i + 1] for i in range(N_CORES // 2)]


def build_nc():
    nc = bacc.Bacc("TRN2", target_bir_lowering=False, debug=False,
                   num_devices=N_CORES)
    xo = nc.dram_tensor("xo", [LH, D], F32, kind="ExternalInput").ap()
    xoTr = nc.dram_tensor("xoTr", [D, LH], F32, kind="ExternalInput").ap()
    wk = nc.dram_tensor("wk", [D, D], F32, kind="ExternalInput").ap()
    wqh = nc.dram_tensor("wqh", [D, DH], F32, kind="ExternalInput").ap()
    wvT = nc.dram_tensor("wvT", [D, D], F32, kind="ExternalInput").ap()
    out = nc.dram_tensor("out", [LH, D], F32, kind="ExternalOutput").ap()
    slots = nc.dram_tensor("slots", [1, 2], mybir.dt.uint32,
                           kind="ExternalInput").ap()
    Gsh = nc.dram_tensor("Gsh", [2, D, D], F32R, addr_space="Shared").ap()
    Hsh = nc.dram_tensor("Hsh", [2, DH, D], F32R, addr_space="Shared").ap()
    tokg = nc.dram_tensor("tokg", [1, 12], F32).ap()
    tokg2 = nc.dram_tensor("tokg2", [1, 12], F32).ap()
    tokh = nc.dram_tensor("tokh", [1, 2], F32).ap()
    tokh2 = nc.dram_tensor("tokh2", [1, 2], F32).ap()
    tokw = nc.dram_tensor("tokw", [1, 2], F32).ap()
    tokw2 = nc.dram_tensor("tokw2", [1, 2], F32).ap()
    tokx = nc.dram_tensor("tokx", [1, 2], F32).ap()
    tokx2 = nc.dram_tensor("tokx2", [1, 2], F32).ap()
    wu_sink = nc.dram_tensor("wu_sink", [P, FH], F32).ap()

    def chunked(ap):  # [K*, N] dram -> [P, K*/P, N] partition-major
        return ap.rearrange("(c p) n -> p c n", p=P)

    with tile.TileContext(nc) as tc, ExitStack() as octx:
        psum = octx.enter_context(tc.tile_pool(name="psum", bufs=8, space="PSUM"))
        # persistent pools: H accumulator (own+peer halves), out staging,
        # tokens + warmup (persistent so nothing WAR-serializes on its space)
        hpool = octx.enter_context(tc.tile_pool(name="hpool", bufs=1))

        tpool = octx.enter_context(tc.tile_pool(name="tpool", bufs=1))
        hf = hpool.tile([P, DC, D], F32R, tag="hf")  # H, own chunks 0-3 peer 4-7

        # dummy AllReduce: pays collective-channel setup and aligns core
        # launch skew while the warmup matmuls run. Issued first so nothing
        # slow sits ahead of it on any queue (everything queued behind a
        # pending collective waits for it).
        wut = tpool.tile([P, FH], F32R, tag="wut")
        nc.vector.memset(wut[:].bitcast(F32), 0.0)
        twu = tpool.tile([1, 2], F32, tag="twu")
        nc.vector.memset(twu[:], 0.0)
        nc.sync.dma_start(tokw[:], twu[:])
        nc.gpsimd.collective_compute(
            "AllReduce", mybir.AluOpType.add, replica_groups=PAIRS,
            ins=[tokw], outs=[tokw2])

        # HAM warmup: junk matmuls while the first DMAs load, so the PE
        # clock gate steps through its ramp (including its ~10us pauses)
        # before real work arrives
        wuo = tpool.tile([P, FH], F32, tag="wuo")
        for g in range(40):
            wp = psum.tile([P, FH], F32, tag="ps", name=f"wu_{g}")
            for r in range(2):
                nc.tensor.matmul(wp[:], wut[:, 0:P], wut[:],
                                 start=(r == 0), stop=(r == 1))
            if g == 39:
                nc.vector.tensor_copy(wuo[:], wp[:])
        nc.sync.dma_start(wu_sink[:], wuo[:])

        # slot selector registers (own / peer) for the shared spill buffers
        st_sl = tpool.tile([1, 2], mybir.dt.uint32, tag="sl")
        nc.sync.dma_start(st_sl[:], slots[:])
        regs_o = nc.alloc_registers(
            engines=[EngineType.SP, EngineType.Activation])
        nc.regs_load(regs_o, st_sl[0:1, 0:1])
        svo = nc.snap(regs_o, donate=True)
        regs_p = nc.alloc_registers(
            engines=[EngineType.SP, EngineType.Activation])
        nc.regs_load(regs_p, st_sl[0:1, 1:2])
        svp = nc.snap(regs_p, donate=True)

        with ExitStack() as gctx:
            # gram-phase pools: G0 + CT + AT (8MB)
            gpool = gctx.enter_context(tc.tile_pool(name="gpool", bufs=1))
            g0 = gpool.tile([P, DC, D], F32R, tag="g0")
            ct = gpool.tile([P, DC, DH], F32R, tag="ct")
            at = gpool.tile([P, DC, DH], F32R, tag="at")

            with ExitStack() as wctx:
                wpool = wctx.enter_context(tc.tile_pool(name="wpool", bufs=1))
                xap = [wctx.enter_context(tc.tile_pool(name=f"xap{i}", bufs=4))
                       for i in range(2)]
                xbp = [wctx.enter_context(tc.tile_pool(name=f"xbp{i}", bufs=3))
                       for i in range(2)]
                wkt = wpool.tile([P, DC, D], F32R, tag="wkt")
                wqt = wpool.tile([P, DC, DH], F32R, tag="wqt")
                nc.sync.dma_start(wkt[:], chunked(wk).bitcast(F32R))
                nc.sync.dma_start(wqt[:], chunked(wqh).bitcast(F32R))

                # x is streamed twice (once per G column-half pass) on two
                # rings. Each stream alternates between two pools in blocks
                # (pool-wrap refills batch on the WHOLE previous wrap, so a
                # single pool starves the PE every wrap), and both streams'
                # DMAs are issued before any matmul so ring order never
                # couples chunk arrival to PE progress.
                def g_stream(pools, block, phase):
                    xcs = []
                    for c in range(XC):
                        pool = pools[(c // block) % 2]
                        xc = pool.tile([P, 1, D], F32R, tag="xc",
                                       name=f"x{phase}_{c}")
                        eng = nc.scalar if c % 2 == 0 else nc.gpsimd
                        eng.dma_start(
                            xc[:], chunked(xo).bitcast(F32R)[:, c:c + 1])
                        xcs.append(xc)
                    return xcs

                def g_pass(xcs, phase, usl):
                    pg = [psum.tile([P, FH], F32, tag="ps",
                                    name=f"g{phase}_{g}") for g in range(DC)]
                    for c in range(XC):
                        src = xcs[c][:, 0]
                        for g in range(DC):
                            nc.tensor.matmul(
                                pg[g][:], src[:, g * P:(g + 1) * P],
                                src[:, usl],
                                start=(c == 0), stop=(c == XC - 1))
                    return pg

                xcs_a = g_stream(xap, 4, "a")
                xcs_b = g_stream(xbp, 3, "b")

                # ---- G pass A (u in [0,512)) ----
                pg = g_pass(xcs_a, "a", slice(0, FH))
                for g in range(DC):
                    nc.vector.tensor_copy(g0[:, g, 0:FH], pg[g][:])
                nc.sync.dma_start(
                    Gsh[bass.ds(svo, 1), :, 0:FH].rearrange(
                        "s (c p) u -> p (s c) u", p=P),
                    g0[:, :, 0:FH])

                # ---- G pass B (u in [512,1024)) ----
                pg = g_pass(xcs_b, "b", slice(FH, D))
                # spill the B-half per row-chunk, alternating rings, so the
                # slow cross-device writes pipeline with the drains
                for g in range(DC):
                    nc.vector.tensor_copy(g0[:, g, FH:D], pg[g][:])
                    eng = nc.sync if g % 2 == 0 else nc.scalar
                    eng.dma_start(
                        Gsh[bass.ds(svo, 1), g * P:(g + 1) * P, FH:D]
                        .rearrange("s (c p) u -> p (s c) u", p=P),
                        g0[:, g:g + 1, FH:D])

                # ---- CT = Wk^T wqh [1024 t, 512 own-d] (hides barrier) ----
                for g in range(DC):
                    pt = psum.tile([P, FH], F32, tag="ps", name=f"ct_{g}")
                    for e in range(DC):
                        nc.tensor.matmul(
                            pt[:], wkt[:, e, g * P:(g + 1) * P], wqt[:, e],
                            start=(e == 0), stop=(e == DC - 1))
                    nc.vector.tensor_copy(ct[:, g], pt[:])

            # ---- pair barrier #1: one token lane per spill piece (RAW dep
            # on each piece's DATA landing in shared DRAM) ----
            tkt = tpool.tile([1, 12], F32, tag="tkt")
            nc.sync.dma_start(tkt[0:1, 0:1], Gsh[bass.ds(svo, 1), 0:1, 0:1]
                              .rearrange("s c n -> c s n").bitcast(F32))
            for g in range(DC):
                nc.sync.dma_start(
                    tkt[0:1, 1 + g:2 + g],
                    Gsh[bass.ds(svo, 1), g * P:g * P + 1, FH:FH + 1]
                    .rearrange("s c n -> c s n").bitcast(F32))
            nc.vector.memset(tkt[0:1, 9:12], 0.0)
            nc.sync.dma_start(tokg[:], tkt[:])
            gbar = nc.gpsimd.collective_compute(
                "AllReduce", mybir.AluOpType.add, replica_groups=PAIRS,
                ins=[tokg], outs=[tokg2])

            with ExitStack() as actx:
                ppool = actx.enter_context(tc.tile_pool(name="ppool", bufs=1))
                g1 = ppool.tile([P, DC, D], F32R, tag="g1")
                wvt = ppool.tile([P, DC, D], F32R, tag="wvt")
                nc.sync.dma_start(wvt[:], chunked(wvT).bitcast(F32R))

                # ---- AT = (G0 + G1) CT: G0 chunks accumulate before the
                # barrier resolves (covering its latency), G1 chunks chase
                # the peer reads ----
                pa = [psum.tile([P, FH], F32, tag="ps", name=f"at_{u}")
                      for u in range(DC)]
                for c in range(DC):
                    for u in range(DC):
                        nc.tensor.matmul(
                            pa[u][:], g0[:, c, u * P:(u + 1) * P], ct[:, c],
                            start=(c == 0), stop=False)
                for c in range(DC):
                    eng = nc.sync if c % 2 == 0 else nc.scalar
                    ld = eng.dma_start(
                        g1[:, c:c + 1],
                        Gsh[bass.ds(svp, 1), c * P:(c + 1) * P, :].rearrange(
                            "s (c p) u -> p (s c) u", p=P))
                    add_dep_helper(ld.ins, gbar.ins,
                                   reason="peer G after pair barrier")
                for c in range(DC):
                    for u in range(DC):
                        nc.tensor.matmul(
                            pa[u][:], g1[:, c, u * P:(u + 1) * P], ct[:, c],
                            start=False, stop=(c == DC - 1))
                for u in range(DC):
                    nc.vector.tensor_copy(at[:, u], pa[u][:])

                # ---- H own half: [512 own-d, 1024 p] -> hf chunks 0-3.
                # Spilled in two pieces pipelined with the compute so the
                # slow cross-device write overlaps the remaining groups ----
                for dt in range(DH // P):
                    for ph in range(D // FH):
                        pt = psum.tile([P, FH], F32, tag="ps",
                                       name=f"h_{dt}_{ph}")
                        for c in range(DC):
                            nc.tensor.matmul(
                                pt[:], at[:, c, dt * P:(dt + 1) * P],
                                wvt[:, c, ph * FH:(ph + 1) * FH],
                                start=(c == 0), stop=(c == DC - 1))
                        nc.vector.tensor_copy(
                            hf[:, dt, ph * FH:(ph + 1) * FH], pt[:])
                    if dt % 2 == 1:
                        half = dt // 2
                        nc.sync.dma_start(
                            Hsh[bass.ds(svo, 1),
                                half * 2 * P:(half + 1) * 2 * P, :].rearrange(
                                "s (c p) n -> p (s c) n", p=P),
                            hf[:, half * 2:half * 2 + 2])

            # ---- pair barrier #2: token sampled from both spill pieces ----
            tkh = tpool.tile([1, 2], F32, tag="tkh")
            nc.sync.dma_start(tkh[0:1, 0:1], Hsh[bass.ds(svo, 1), 0:1, 0:1]
                              .rearrange("s c n -> c s n").bitcast(F32))
            nc.sync.dma_start(tkh[0:1, 1:2],
                              Hsh[bass.ds(svo, 1), 256:257, 0:1]
                              .rearrange("s c n -> c s n").bitcast(F32))
            nc.sync.dma_start(tokh[:], tkh[:])
            hbar = nc.gpsimd.collective_compute(
                "AllReduce", mybir.AluOpType.add, replica_groups=PAIRS,
                ins=[tokh], outs=[tokh2])
            for c in range(DH // P):
                ld = nc.sync.dma_start(
                    hf[:, DH // P + c:DH // P + c + 1],
                    Hsh[bass.ds(svp, 1), c * P:(c + 1) * P, :].rearrange(
                        "s (c p) n -> p (s c) n", p=P))
                add_dep_helper(ld.ins, hbar.ins,
                               reason="peer H after pair barrier")

        # ---- out = xoTr^T hf (d rotated own-first on host). Two passes:
        # the own-H half-contraction for ALL row tiles first (so the full
        # 35us of own-H work covers barrier #2 + peer-H latency despite the
        # in-order tensor queue), then the peer-H half added in. ----
        with ExitStack() as bctx:
            xtpool = bctx.enter_context(tc.tile_pool(name="xtpool", bufs=1))
            obpool = bctx.enter_context(tc.tile_pool(name="obpool", bufs=1))
            xt = xtpool.tile([P, DC, LH], F32R, tag="xt")
            ob = obpool.tile([P, LH // P, D], F32, tag="ob")
            # all on the scalar ring: gpsimd/sync still have the pending
            # barrier collective + peer reads queued ahead at this point
            for c in range(DC):
                nc.scalar.dma_start(xt[:, c:c + 1],
                                    chunked(xoTr).bitcast(F32R)[:, c:c + 1])

            HC = DC // 2
            for lt in range(LH // P):
                for ph in range(D // FH):
                    pt = psum.tile([P, FH], F32, tag="ps",
                                   name=f"oa_{lt}_{ph}")
                    for c in range(HC):
                        nc.tensor.matmul(
                            pt[:], xt[:, c, lt * P:(lt + 1) * P],
                            hf[:, c, ph * FH:(ph + 1) * FH],
                            start=(c == 0), stop=(c == HC - 1))
                    nc.vector.tensor_copy(
                        ob[:, lt, ph * FH:(ph + 1) * FH], pt[:])
            for lt in range(LH // P):
                for ph in range(D // FH):
                    pt = psum.tile([P, FH], F32, tag="ps",
                                   name=f"ob_{lt}_{ph}")
                    for c in range(HC, DC):
                        nc.tensor.matmul(
                            pt[:], xt[:, c, lt * P:(lt + 1) * P],
                            hf[:, c, ph * FH:(ph + 1) * FH],
                            start=(c == HC), stop=(c == DC - 1))
                    nc.vector.tensor_add(
                        ob[:, lt, ph * FH:(ph + 1) * FH],
                        ob[:, lt, ph * FH:(ph + 1) * FH], pt[:])
                nc.scalar.dma_start(out[lt * P:(lt + 1) * P, :], ob[:, lt])

    nc.compile()
    return nc


_NC_CACHE = {}


def _get_nc():
    if "nc" not in _NC_CACHE:
        _NC_CACHE["nc"] = build_nc()
    return _NC_CACHE["nc"]


def run(inputs, trace=False):
    """Run the kernel on all 8 cores. Returns (full_output, BassKernelResults)."""
    x = np.asarray(inputs["x"], dtype=np.float32)
    Wq = np.asarray(inputs["Wq"], dtype=np.float32)
    Wk = np.asarray(inputs["Wk"], dtype=np.float32)
    Wv = np.asarray(inputs["Wv"], dtype=np.float32)

    inv_sqrt_d = np.float32(1.0 / np.sqrt(D))
    wvT = np.ascontiguousarray(Wv.T)

    in_maps = []
    for c in range(N_CORES):
        b, h = c // 2, c % 2
        xb = x[b, h * LH:(h + 1) * LH, :]          # [2048, 1024]
        xbT = xb.T                                  # [1024, 2048]
        # rotate d-rows of xoT so this core's own d-half comes first
        xoTr = np.concatenate(
            [xbT[h * DH:(h + 1) * DH], xbT[(1 - h) * DH:(2 - h) * DH]],
            axis=0)
        in_maps.append({
            "xo": np.ascontiguousarray(xb),
            "xoTr": np.ascontiguousarray(xoTr),
            "wk": Wk,
            "wqh": np.ascontiguousarray(Wq[:, h * DH:(h + 1) * DH]
                                        * inv_sqrt_d),
            "wvT": wvT,
            "slots": np.array([[h, 1 - h]], dtype=np.uint32),
        })

    nc = _get_nc()
    res = run_bass_kernel_spmd(nc, in_maps, list(range(N_CORES)), trace=trace)

    full = np.empty((B, L, D), dtype=np.float32)
    for c in range(N_CORES):
        b, h = c // 2, c % 2
        full[b, h * LH:(h + 1) * LH, :] = res.results[c]["out"]
    return full, res


def kernel(**inputs):
    full, _ = run(inputs, trace=False)
    return full


# revision 37
# speedup vs baseline: 1.0393x; 1.0393x over previous
"""Trainium2 Bass kernel for softmax-free attention:
    q = x @ Wq^T; k = x @ Wk^T; v = x @ Wv^T
    s = (q @ k^T) / sqrt(d); out = s @ v
  x: [4, 4096, 1024], W*: [1024, 1024], out: [4, 4096, 1024] (fp32)

There is no softmax, so the attention is linear and reassociates:
    out = x . (Wq^T Wk) . (x^T x) . Wv^T / sqrt(d)
which replaces the two [L,L]-score matmuls (2*L^2*D flops/batch) with a
[D,D] Gram-matrix chain (2*L*D^2 + small), a ~4.3x flop reduction.

Sharding: 8 cores; core c handles batch c//2, row-half c%2 (2048 rows).
Per core:
  G0    = xo^T xo                  (own-row Gram partial, [1024,1024])
  CT    = Wk^T wqh                 (wqh = Wq[:, own 512 d-cols]/sqrt(d))
  AT    = (G0 + G1) CT             (G1 = partner Gram via shared DRAM)
  Hown  = AT^T WvT                 ([512,1024] own-d rows of H)
  out   = xoT^T [Hown; Hpeer]      (partner H half via shared DRAM)
The pair exchanges G0 (4MB) and H halves (2MB) through cross-core-visible
Shared-DRAM slots; ordering is a token AllReduce whose token is DMA-sampled
from the shared buffers (RAW dep on the spill writes). The AT accumulation
runs G0's 8 chunks before the barrier and G1's 8 chunks after, hiding the
barrier latency; the out accumulation orders own-H chunks first for the same
reason. Host-side, xoT rows are rotated own-d-half-first so every tile index
is static across the SPMD program (only the shared-DRAM slot is dynamic).

All matmul inputs are float32r (full PE rate at free-dim>=256); 1/sqrt(d)
is folded into wqh on the host.
"""

import sys
import types
from contextlib import ExitStack

import numpy as np

import concourse.bass as bass
import concourse.tile as tile
from concourse import bacc, mybir
from concourse.bass_utils import run_bass_kernel_spmd
from concourse.mybir import EngineType
from concourse.tile import add_dep_helper
from concourse.vector_clock import ScopedClock

# ---------------------------------------------------------------------------
# Environment shims
# ---------------------------------------------------------------------------


def _install_tile_drain_patch():
    """This toolchain's walrus caps sync waits at 1 per instruction, but
    TileContext's tail drain can carry several. Split the overflow onto
    preceding nops (same semantics: the issuing engine observes every sem
    before draining)."""
    if getattr(tile.TileContext, "_drain_patch_installed", False):
        return

    def _patched_drain_and_barrier(self, tick_clock, wait_clock):
        nc = self.nc
        collector = nc.sync.nop(hint="drain_wait_collector", nofuse=True)
        wait_clock.add_sem_waits(
            collector.ins, ScopedClock({None: tick_clock.global_clock})
        )
        waits = list(collector.ins.sync_info.on_wait or [])
        if len(waits) > 1:
            collector.ins.sync_info.on_wait = [waits[0]]
            for w in waits[1:]:
                nop = nc.sync.nop(hint="drain_wait_extra", nofuse=True)
                nop.ins.sync_info = mybir.SyncInfo(on_wait=[w], on_update=[])
        nc.sync.drain()

        nc.all_engine_barrier()
        assert self.sems is not None
        popped = nc._tile_sem_poison_stack.pop()
        assert popped is self._sem_poison
        nc.clear_and_free_semaphores(list(self.sems.allocated().values()))
        nc.all_engine_barrier()

    tile.TileContext._drain_and_barrier = _patched_drain_and_barrier
    tile.TileContext._drain_patch_installed = True


def _install_ntff_shim():
    """The image's antenv lacks axon_hooks, which silently degrades
    trace=True. Recreate the get/set pair and register the ctypes NTFF hook
    from trn_agent_boot (no-op if unavailable)."""
    if "antenv.axon_hooks" in sys.modules:
        return
    state = {"hook": None}

    def set_axon_ntff_profile_hook(h):
        state["hook"] = h

    def get_axon_ntff_profile_hook():
        return state["hook"]

    mod = types.ModuleType("antenv.axon_hooks")
    mod.set_axon_ntff_profile_hook = set_axon_ntff_profile_hook
    mod.get_axon_ntff_profile_hook = get_axon_ntff_profile_hook
    sys.modules["antenv.axon_hooks"] = mod
    try:
        import antenv

        antenv.axon_hooks = mod
        from trn_agent_boot.trn_boot import _ntff_profile_via_ctypes

        set_axon_ntff_profile_hook(
            _ntff_profile_via_ctypes("/opt/axon/libaxon_pjrt.so")
        )
    except Exception:
        pass


_install_tile_drain_patch()
_install_ntff_shim()

# ---------------------------------------------------------------------------
# Problem constants (hardcoded per the harness contract)
# ---------------------------------------------------------------------------

B, L, D = 4, 4096, 1024
N_CORES = 8
P = 128
LH = L // 2       # rows per core
DC = D // P       # 8 chunks of 128 over d/e/t/u
XC = LH // P      # 16 row-chunks of own x
DH = D // 2       # 512: own d-half width
F32 = mybir.dt.float32
F32R = mybir.dt.float32r
FH = 512          # matmul free-dim / psum group width

PAIRS = [[2 * i, 2 * i + 1] for i in range(N_CORES // 2)]


def build_nc():
    nc = bacc.Bacc("TRN2", target_bir_lowering=False, debug=False,
                   num_devices=N_CORES)
    xo = nc.dram_tensor("xo", [LH, D], F32, kind="ExternalInput").ap()
    xoTr = nc.dram_tensor("xoTr", [D, LH], F32, kind="ExternalInput").ap()
    wk = nc.dram_tensor("wk", [D, D], F32, kind="ExternalInput").ap()
    wqh = nc.dram_tensor("wqh", [D, DH], F32, kind="ExternalInput").ap()
    wvT = nc.dram_tensor("wvT", [D, D], F32, kind="ExternalInput").ap()
    out = nc.dram_tensor("out", [LH, D], F32, kind="ExternalOutput").ap()
    slots = nc.dram_tensor("slots", [1, 2], mybir.dt.uint32,
                           kind="ExternalInput").ap()
    Gsh = nc.dram_tensor("Gsh", [2, D, D], F32R, addr_space="Shared").ap()
    Hsh = nc.dram_tensor("Hsh", [2, DH, D], F32R, addr_space="Shared").ap()
    tokg = nc.dram_tensor("tokg", [1, 4], F32).ap()
    tokg2 = nc.dram_tensor("tokg2", [1, 4], F32).ap()
    tokh = nc.dram_tensor("tokh", [1, 2], F32).ap()
    tokh2 = nc.dram_tensor("tokh2", [1, 2], F32).ap()
    tokw = nc.dram_tensor("tokw", [1, 2], F32).ap()
    tokw2 = nc.dram_tensor("tokw2", [1, 2], F32).ap()
    tokx = nc.dram_tensor("tokx", [1, 2], F32).ap()
    tokx2 = nc.dram_tensor("tokx2", [1, 2], F32).ap()
    wu_sink = nc.dram_tensor("wu_sink", [P, 256], F32).ap()

    def chunked(ap):  # [K*, N] dram -> [P, K*/P, N] partition-major
        return ap.rearrange("(c p) n -> p c n", p=P)

    with tile.TileContext(nc) as tc, ExitStack() as octx:
        psum = octx.enter_context(tc.tile_pool(name="psum", bufs=8, space="PSUM"))
        # persistent pools: H accumulator (own+peer halves), out staging,
        # tokens + warmup (persistent so nothing WAR-serializes on its space)
        hpool = octx.enter_context(tc.tile_pool(name="hpool", bufs=1))

        tpool = octx.enter_context(tc.tile_pool(name="tpool", bufs=1))
        hf = hpool.tile([P, DC, D], F32R, tag="hf")  # H, own chunks 0-3 peer 4-7

        # dummy AllReduce: pays collective-channel setup and aligns core
        # launch skew while the warmup matmuls run. Issued first so nothing
        # slow sits ahead of it on any queue (everything queued behind a
        # pending collective waits for it).
        wut = tpool.tile([P, 256], F32R, tag="wut")
        nc.vector.memset(wut[:].bitcast(F32), 0.0)
        twu = tpool.tile([1, 2], F32, tag="twu")
        nc.vector.memset(twu[:], 0.0)
        nc.sync.dma_start(tokw[:], twu[:])
        nc.gpsimd.collective_compute(
            "AllReduce", mybir.AluOpType.add, replica_groups=PAIRS,
            ins=[tokw], outs=[tokw2])

        # HAM warmup: junk matmuls while the first DMAs load, so the PE
        # clock gate steps through its ramp (including its ~10us pauses)
        # before real work arrives
        wuo = tpool.tile([P, 256], F32, tag="wuo")
        for g in range(40):
            wp = psum.tile([P, 256], F32, tag="ps", name=f"wu_{g}")
            for r in range(2):
                nc.tensor.matmul(wp[:], wut[:, 0:P], wut[:],
                                 start=(r == 0), stop=(r == 1))
            if g == 39:
                nc.vector.tensor_copy(wuo[:], wp[:])
        nc.sync.dma_start(wu_sink[:], wuo[:])

        # slot selector registers (own / peer) for the shared spill buffers
        st_sl = tpool.tile([1, 2], mybir.dt.uint32, tag="sl")
        nc.sync.dma_start(st_sl[:], slots[:])
        regs_o = nc.alloc_registers(
            engines=[EngineType.SP, EngineType.Activation])
        nc.regs_load(regs_o, st_sl[0:1, 0:1])
        svo = nc.snap(regs_o, donate=True)
        regs_p = nc.alloc_registers(
            engines=[EngineType.SP, EngineType.Activation])
        nc.regs_load(regs_p, st_sl[0:1, 1:2])
        svp = nc.snap(regs_p, donate=True)

        with ExitStack() as gctx:
            # gram-phase pools: G0 + CT + AT (8MB)
            gpool = gctx.enter_context(tc.tile_pool(name="gpool", bufs=1))
            g0 = gpool.tile([P, DC, D], F32R, tag="g0")
            ct = gpool.tile([P, DC, DH], F32R, tag="ct")
            at = gpool.tile([P, DC, DH], F32R, tag="at")

            with ExitStack() as wctx:
                wpool = wctx.enter_context(tc.tile_pool(name="wpool", bufs=1))
                xap = [wctx.enter_context(tc.tile_pool(name=f"xap{i}", bufs=4))
                       for i in range(2)]
                xbp = [wctx.enter_context(tc.tile_pool(name=f"xbp{i}", bufs=3))
                       for i in range(2)]
                wkt = wpool.tile([P, DC, D], F32R, tag="wkt")
                wqt = wpool.tile([P, DC, DH], F32R, tag="wqt")
                nc.sync.dma_start(wkt[:], chunked(wk).bitcast(F32R))
                nc.sync.dma_start(wqt[:], chunked(wqh).bitcast(F32R))

                # x is streamed twice (once per G column-half pass) on two
                # rings. Each stream alternates between two pools in blocks
                # (pool-wrap refills batch on the WHOLE previous wrap, so a
                # single pool starves the PE every wrap), and both streams'
                # DMAs are issued before any matmul so ring order never
                # couples chunk arrival to PE progress.
                def g_stream(pools, block, phase):
                    xcs = []
                    for c in range(XC):
                        pool = pools[(c // block) % 2]
                        xc = pool.tile([P, 1, D], F32R, tag="xc",
                                       name=f"x{phase}_{c}")
                        eng = nc.scalar if c % 2 == 0 else nc.gpsimd
                        eng.dma_start(
                            xc[:], chunked(xo).bitcast(F32R)[:, c:c + 1])
                        xcs.append(xc)
                    return xcs

                def g_pass(xcs, phase, usl):
                    pg = [psum.tile([P, FH], F32, tag="ps",
                                    name=f"g{phase}_{g}") for g in range(DC)]
                    for c in range(XC):
                        src = xcs[c][:, 0]
                        for g in range(DC):
                            nc.tensor.matmul(
                                pg[g][:], src[:, g * P:(g + 1) * P],
                                src[:, usl],
                                start=(c == 0), stop=(c == XC - 1))
                    return pg

                xcs_a = g_stream(xap, 4, "a")
                xcs_b = g_stream(xbp, 3, "b")

                # ---- G pass A (u in [0,512)) ----
                pg = g_pass(xcs_a, "a", slice(0, FH))
                for g in range(DC):
                    nc.vector.tensor_copy(g0[:, g, 0:FH], pg[g][:])
                nc.sync.dma_start(
                    Gsh[bass.ds(svo, 1), :, 0:FH].rearrange(
                        "s (c p) u -> p (s c) u", p=P),
                    g0[:, :, 0:FH])

                # ---- G pass B (u in [512,1024)) ----
                pg = g_pass(xcs_b, "b", slice(FH, D))
                # spill the B-half as two 1MB pieces on two rings: the
                # cross-device writes run ~50-100GB/s per ring, and pieces
                # smaller than ~1MB lose to per-piece latency
                for g in range(DC):
                    nc.vector.tensor_copy(g0[:, g, FH:D], pg[g][:])
                for half in range(2):
                    eng = nc.sync if half == 0 else nc.scalar
                    r0 = half * (DC // 2)
                    eng.dma_start(
                        Gsh[bass.ds(svo, 1), r0 * P:(r0 + DC // 2) * P, FH:D]
                        .rearrange("s (c p) u -> p (s c) u", p=P),
                        g0[:, r0:r0 + DC // 2, FH:D])

                # ---- CT = Wk^T wqh [1024 t, 512 own-d] (hides barrier) ----
                for g in range(DC):
                    pt = psum.tile([P, FH], F32, tag="ps", name=f"ct_{g}")
                    for e in range(DC):
                        nc.tensor.matmul(
                            pt[:], wkt[:, e, g * P:(g + 1) * P], wqt[:, e],
                            start=(e == 0), stop=(e == DC - 1))
                    nc.vector.tensor_copy(ct[:, g], pt[:])

            # ---- pair barrier #1: one token lane per spill piece (RAW dep
            # on each piece's DATA landing in shared DRAM) ----
            tkt = tpool.tile([1, 4], F32, tag="tkt")
            nc.sync.dma_start(tkt[0:1, 0:1], Gsh[bass.ds(svo, 1), 0:1, 0:1]
                              .rearrange("s c n -> c s n").bitcast(F32))
            for half in range(2):
                r0 = half * (DC // 2) * P
                nc.sync.dma_start(
                    tkt[0:1, 1 + half:2 + half],
                    Gsh[bass.ds(svo, 1), r0:r0 + 1, FH:FH + 1]
                    .rearrange("s c n -> c s n").bitcast(F32))
            nc.vector.memset(tkt[0:1, 3:4], 0.0)
            nc.sync.dma_start(tokg[:], tkt[:])
            gbar = nc.gpsimd.collective_compute(
                "AllReduce", mybir.AluOpType.add, replica_groups=PAIRS,
                ins=[tokg], outs=[tokg2])

            with ExitStack() as actx:
                ppool = actx.enter_context(tc.tile_pool(name="ppool", bufs=1))
                g1 = ppool.tile([P, DC, D], F32R, tag="g1")
                wvt = ppool.tile([P, DC, D], F32R, tag="wvt")
                nc.sync.dma_start(wvt[:], chunked(wvT).bitcast(F32R))

                # ---- AT = (G0 + G1) CT: G0 chunks accumulate before the
                # barrier resolves (covering its latency), G1 chunks chase
                # the peer reads ----
                pa = [psum.tile([P, FH], F32, tag="ps", name=f"at_{u}")
                      for u in range(DC)]
                for c in range(DC):
                    for u in range(DC):
                        nc.tensor.matmul(
                            pa[u][:], g0[:, c, u * P:(u + 1) * P], ct[:, c],
                            start=(c == 0), stop=False)
                for c in range(DC):
                    eng = nc.sync if c % 2 == 0 else nc.scalar
                    ld = eng.dma_start(
                        g1[:, c:c + 1],
                        Gsh[bass.ds(svp, 1), c * P:(c + 1) * P, :].rearrange(
                            "s (c p) u -> p (s c) u", p=P))
                    add_dep_helper(ld.ins, gbar.ins,
                                   reason="peer G after pair barrier")
                for c in range(DC):
                    for u in range(DC):
                        nc.tensor.matmul(
                            pa[u][:], g1[:, c, u * P:(u + 1) * P], ct[:, c],
                            start=False, stop=(c == DC - 1))
                for u in range(DC):
                    nc.vector.tensor_copy(at[:, u], pa[u][:])

                # ---- H own half: [512 own-d, 1024 p] -> hf chunks 0-3.
                # Spilled in two pieces pipelined with the compute so the
                # slow cross-device write overlaps the remaining groups ----
                for dt in range(DH // P):
                    for ph in range(D // FH):
                        pt = psum.tile([P, FH], F32, tag="ps",
                                       name=f"h_{dt}_{ph}")
                        for c in range(DC):
                            nc.tensor.matmul(
                                pt[:], at[:, c, dt * P:(dt + 1) * P],
                                wvt[:, c, ph * FH:(ph + 1) * FH],
                                start=(c == 0), stop=(c == DC - 1))
                        nc.vector.tensor_copy(
                            hf[:, dt, ph * FH:(ph + 1) * FH], pt[:])
                    if dt % 2 == 1:
                        half = dt // 2
                        nc.sync.dma_start(
                            Hsh[bass.ds(svo, 1),
                                half * 2 * P:(half + 1) * 2 * P, :].rearrange(
                                "s (c p) n -> p (s c) n", p=P),
                            hf[:, half * 2:half * 2 + 2])

            # ---- pair barrier #2: token sampled from both spill pieces ----
            tkh = tpool.tile([1, 2], F32, tag="tkh")
            nc.sync.dma_start(tkh[0:1, 0:1], Hsh[bass.ds(svo, 1), 0:1, 0:1]
                              .rearrange("s c n -> c s n").bitcast(F32))
            nc.sync.dma_start(tkh[0:1, 1:2],
                              Hsh[bass.ds(svo, 1), 256:257, 0:1]
                              .rearrange("s c n -> c s n").bitcast(F32))
            nc.sync.dma_start(tokh[:], tkh[:])
            hbar = nc.gpsimd.collective_compute(
                "AllReduce", mybir.AluOpType.add, replica_groups=PAIRS,
                ins=[tokh], outs=[tokh2])
            for c in range(DH // P):
                ld = nc.sync.dma_start(
                    hf[:, DH // P + c:DH // P + c + 1],
                    Hsh[bass.ds(svp, 1), c * P:(c + 1) * P, :].rearrange(
                        "s (c p) n -> p (s c) n", p=P))
                add_dep_helper(ld.ins, hbar.ins,
                               reason="peer H after pair barrier")

        # ---- out = xoTr^T hf (d rotated own-first on host). Two passes:
        # the own-H half-contraction for ALL row tiles first (so the full
        # 35us of own-H work covers barrier #2 + peer-H latency despite the
        # in-order tensor queue), then the peer-H half added in. ----
        with ExitStack() as bctx:
            xtpool = bctx.enter_context(tc.tile_pool(name="xtpool", bufs=1))
            obpool = bctx.enter_context(tc.tile_pool(name="obpool", bufs=1))
            xt = xtpool.tile([P, DC, LH], F32R, tag="xt")
            ob = obpool.tile([P, LH // P, D], F32, tag="ob")
            # all on the scalar ring: gpsimd/sync still have the pending
            # barrier collective + peer reads queued ahead at this point
            for c in range(DC):
                nc.scalar.dma_start(xt[:, c:c + 1],
                                    chunked(xoTr).bitcast(F32R)[:, c:c + 1])

            HC = DC // 2
            for lt in range(LH // P):
                for ph in range(D // FH):
                    pt = psum.tile([P, FH], F32, tag="ps",
                                   name=f"oa_{lt}_{ph}")
                    for c in range(HC):
                        nc.tensor.matmul(
                            pt[:], xt[:, c, lt * P:(lt + 1) * P],
                            hf[:, c, ph * FH:(ph + 1) * FH],
                            start=(c == 0), stop=(c == HC - 1))
                    nc.vector.tensor_copy(
                        ob[:, lt, ph * FH:(ph + 1) * FH], pt[:])
            for lt in range(LH // P):
                for ph in range(D // FH):
                    pt = psum.tile([P, FH], F32, tag="ps",
                                   name=f"ob_{lt}_{ph}")
                    for c in range(HC, DC):
                        nc.tensor.matmul(
                            pt[:], xt[:, c, lt * P:(lt + 1) * P],
                            hf[:, c, ph * FH:(ph + 1) * FH],
                            start=(c == HC), stop=(c == DC - 1))
                    nc.vector.tensor_add(
                        ob[:, lt, ph * FH:(ph + 1) * FH],
                        ob[:, lt, ph * FH:(ph + 1) * FH], pt[:])
                nc.scalar.dma_start(out[lt * P:(lt + 1) * P, :], ob[:, lt])

    nc.compile()
    return nc


_NC_CACHE = {}


def _get_nc():
    if "nc" not in _NC_CACHE:
        _NC_CACHE["nc"] = build_nc()
    return _NC_CACHE["nc"]


def run(inputs, trace=False):
    """Run the kernel on all 8 cores. Returns (full_output, BassKernelResults)."""
    x = np.asarray(inputs["x"], dtype=np.float32)
    Wq = np.asarray(inputs["Wq"], dtype=np.float32)
    Wk = np.asarray(inputs["Wk"], dtype=np.float32)
    Wv = np.asarray(inputs["Wv"], dtype=np.float32)

    inv_sqrt_d = np.float32(1.0 / np.sqrt(D))
    wvT = np.ascontiguousarray(Wv.T)

    in_maps = []
    for c in range(N_CORES):
        b, h = c // 2, c % 2
        xb = x[b, h * LH:(h + 1) * LH, :]          # [2048, 1024]
        xbT = xb.T                                  # [1024, 2048]
        # rotate d-rows of xoT so this core's own d-half comes first
        xoTr = np.concatenate(
            [xbT[h * DH:(h + 1) * DH], xbT[(1 - h) * DH:(2 - h) * DH]],
            axis=0)
        in_maps.append({
            "xo": np.ascontiguousarray(xb),
            "xoTr": np.ascontiguousarray(xoTr),
            "wk": Wk,
            "wqh": np.ascontiguousarray(Wq[:, h * DH:(h + 1) * DH]
                                        * inv_sqrt_d),
            "wvT": wvT,
            "slots": np.array([[h, 1 - h]], dtype=np.uint32),
        })

    nc = _get_nc()
    res = run_bass_kernel_spmd(nc, in_maps, list(range(N_CORES)), trace=trace)

    full = np.empty((B, L, D), dtype=np.float32)
    for c in range(N_CORES):
        b, h = c // 2, c % 2
        full[b, h * LH:(h + 1) * LH, :] = res.results[c]["out"]
    return full, res


def kernel(**inputs):
    full, _ = run(inputs, trace=False)
    return full


# revision 40
# speedup vs baseline: 1.1500x; 1.1066x over previous
"""Trainium2 Bass kernel for softmax-free attention:
    q = x @ Wq^T; k = x @ Wk^T; v = x @ Wv^T
    s = (q @ k^T) / sqrt(d); out = s @ v
  x: [4, 4096, 1024], W*: [1024, 1024], out: [4, 4096, 1024] (fp32)

There is no softmax, so the attention is linear and reassociates:
    out = x . (Wq^T Wk) . (x^T x) . Wv^T / sqrt(d)
which replaces the two [L,L]-score matmuls (2*L^2*D flops/batch) with a
[D,D] Gram-matrix chain (2*L*D^2 + small), a ~4.3x flop reduction.

Sharding: 8 cores; core c handles batch c//2, row-half c%2 (2048 rows).
Per core:
  G0    = xo^T xo                  (own-row Gram partial, [1024,1024])
  CT    = Wk^T wqh                 (wqh = Wq[:, own 512 d-cols]/sqrt(d))
  AT    = (G0 + G1) CT             (G1 = partner Gram via shared DRAM)
  Hown  = AT^T WvT                 ([512,1024] own-d rows of H)
  out   = xoT^T [Hown; Hpeer]      (partner H half via shared DRAM)
The pair exchanges G0 (4MB) and H halves (2MB) through cross-core-visible
Shared-DRAM slots; ordering is a token AllReduce whose token is DMA-sampled
from the shared buffers (RAW dep on the spill writes). The AT accumulation
runs G0's 8 chunks before the barrier and G1's 8 chunks after, hiding the
barrier latency; the out accumulation orders own-H chunks first for the same
reason. Host-side, xoT rows are rotated own-d-half-first so every tile index
is static across the SPMD program (only the shared-DRAM slot is dynamic).

All matmul inputs are float32r (full PE rate at free-dim>=256); 1/sqrt(d)
is folded into wqh on the host.
"""

import sys
import types
from contextlib import ExitStack

import numpy as np

import concourse.bass as bass
import concourse.tile as tile
from concourse import bacc, mybir
from concourse.bass_utils import run_bass_kernel_spmd
from concourse.mybir import EngineType
from concourse.tile import add_dep_helper
from concourse.vector_clock import ScopedClock

# ---------------------------------------------------------------------------
# Environment shims
# ---------------------------------------------------------------------------


def _install_tile_drain_patch():
    """This toolchain's walrus caps sync waits at 1 per instruction, but
    TileContext's tail drain can carry several. Split the overflow onto
    preceding nops (same semantics: the issuing engine observes every sem
    before draining)."""
    if getattr(tile.TileContext, "_drain_patch_installed", False):
        return

    def _patched_drain_and_barrier(self, tick_clock, wait_clock):
        nc = self.nc
        collector = nc.sync.nop(hint="drain_wait_collector", nofuse=True)
        wait_clock.add_sem_waits(
            collector.ins, ScopedClock({None: tick_clock.global_clock})
        )
        waits = list(collector.ins.sync_info.on_wait or [])
        if len(waits) > 1:
            collector.ins.sync_info.on_wait = [waits[0]]
            for w in waits[1:]:
                nop = nc.sync.nop(hint="drain_wait_extra", nofuse=True)
                nop.ins.sync_info = mybir.SyncInfo(on_wait=[w], on_update=[])
        nc.sync.drain()

        nc.all_engine_barrier()
        assert self.sems is not None
        popped = nc._tile_sem_poison_stack.pop()
        assert popped is self._sem_poison
        nc.clear_and_free_semaphores(list(self.sems.allocated().values()))
        nc.all_engine_barrier()

    tile.TileContext._drain_and_barrier = _patched_drain_and_barrier
    tile.TileContext._drain_patch_installed = True


def _install_ntff_shim():
    """The image's antenv lacks axon_hooks, which silently degrades
    trace=True. Recreate the get/set pair and register the ctypes NTFF hook
    from trn_agent_boot (no-op if unavailable)."""
    if "antenv.axon_hooks" in sys.modules:
        return
    state = {"hook": None}

    def set_axon_ntff_profile_hook(h):
        state["hook"] = h

    def get_axon_ntff_profile_hook():
        return state["hook"]

    mod = types.ModuleType("antenv.axon_hooks")
    mod.set_axon_ntff_profile_hook = set_axon_ntff_profile_hook
    mod.get_axon_ntff_profile_hook = get_axon_ntff_profile_hook
    sys.modules["antenv.axon_hooks"] = mod
    try:
        import antenv

        antenv.axon_hooks = mod
        from trn_agent_boot.trn_boot import _ntff_profile_via_ctypes

        set_axon_ntff_profile_hook(
            _ntff_profile_via_ctypes("/opt/axon/libaxon_pjrt.so")
        )
    except Exception:
        pass


_install_tile_drain_patch()
_install_ntff_shim()

# ---------------------------------------------------------------------------
# Problem constants (hardcoded per the harness contract)
# ---------------------------------------------------------------------------

B, L, D = 4, 4096, 1024
N_CORES = 8
P = 128
LH = L // 2       # rows per core
DC = D // P       # 8 chunks of 128 over d/e/t/u
XC = LH // P      # 16 row-chunks of own x
DH = D // 2       # 512: own d-half width
F32 = mybir.dt.float32
F32R = mybir.dt.float32r
FH = 512          # matmul free-dim / psum group width

PAIRS = [[2 * i, 2 * i + 1] for i in range(N_CORES // 2)]


def build_nc():
    nc = bacc.Bacc("TRN2", target_bir_lowering=False, debug=False,
                   num_devices=N_CORES)
    xo = nc.dram_tensor("xo", [LH, D], F32, kind="ExternalInput").ap()
    xoTr = nc.dram_tensor("xoTr", [D, LH], F32, kind="ExternalInput").ap()
    wk = nc.dram_tensor("wk", [D, D], F32, kind="ExternalInput").ap()
    wqh = nc.dram_tensor("wqh", [D, DH], F32, kind="ExternalInput").ap()
    wvT = nc.dram_tensor("wvT", [D, D], F32, kind="ExternalInput").ap()
    out = nc.dram_tensor("out", [LH, D], F32, kind="ExternalOutput").ap()
    slots = nc.dram_tensor("slots", [1, 2], mybir.dt.uint32,
                           kind="ExternalInput").ap()
    Gsh = nc.dram_tensor("Gsh", [2, D, D], F32R, addr_space="Shared").ap()
    Hsh = nc.dram_tensor("Hsh", [2, DH, D], F32R, addr_space="Shared").ap()
    tokg = nc.dram_tensor("tokg", [1, 4], F32).ap()
    tokg2 = nc.dram_tensor("tokg2", [1, 4], F32).ap()
    tokh = nc.dram_tensor("tokh", [1, 2], F32).ap()
    tokh2 = nc.dram_tensor("tokh2", [1, 2], F32).ap()
    tokw = nc.dram_tensor("tokw", [1, 2], F32).ap()
    tokw2 = nc.dram_tensor("tokw2", [1, 2], F32).ap()
    tokx = nc.dram_tensor("tokx", [1, 2], F32).ap()
    tokx2 = nc.dram_tensor("tokx2", [1, 2], F32).ap()
    wu_sink = nc.dram_tensor("wu_sink", [P, 256], F32).ap()

    def chunked(ap):  # [K*, N] dram -> [P, K*/P, N] partition-major
        return ap.rearrange("(c p) n -> p c n", p=P)

    with tile.TileContext(nc) as tc, ExitStack() as octx:
        psum = octx.enter_context(tc.tile_pool(name="psum", bufs=8, space="PSUM"))
        # persistent pools: H accumulator (own+peer halves), out staging,
        # tokens + warmup (persistent so nothing WAR-serializes on its space)
        hpool = octx.enter_context(tc.tile_pool(name="hpool", bufs=1))

        tpool = octx.enter_context(tc.tile_pool(name="tpool", bufs=1))
        hf = hpool.tile([P, DC, D], F32R, tag="hf")  # H, own chunks 0-3 peer 4-7

        # dummy AllReduce: pays collective-channel setup and aligns core
        # launch skew while the warmup matmuls run. Issued first so nothing
        # slow sits ahead of it on any queue (everything queued behind a
        # pending collective waits for it).
        wut = tpool.tile([P, 256], F32R, tag="wut")
        nc.vector.memset(wut[:].bitcast(F32), 0.0)
        twu = tpool.tile([1, 2], F32, tag="twu")
        nc.vector.memset(twu[:], 0.0)
        nc.sync.dma_start(tokw[:], twu[:])
        nc.gpsimd.collective_compute(
            "AllReduce", mybir.AluOpType.add, replica_groups=PAIRS,
            ins=[tokw], outs=[tokw2])

        # HAM warmup: junk matmuls while the first DMAs load, so the PE
        # clock gate steps through its ramp (including its ~10us pauses)
        # before real work arrives
        wuo = tpool.tile([P, 256], F32, tag="wuo")
        for g in range(80):
            wp = psum.tile([P, 256], F32, tag="ps", name=f"wu_{g}")
            for r in range(2):
                nc.tensor.matmul(wp[:], wut[:, 0:P], wut[:],
                                 start=(r == 0), stop=(r == 1))
            if g == 79:
                nc.vector.tensor_copy(wuo[:], wp[:])
        nc.sync.dma_start(wu_sink[:], wuo[:])

        # slot selector registers (own / peer) for the shared spill buffers
        st_sl = tpool.tile([1, 2], mybir.dt.uint32, tag="sl")
        nc.sync.dma_start(st_sl[:], slots[:])
        regs_o = nc.alloc_registers(
            engines=[EngineType.SP, EngineType.Activation])
        nc.regs_load(regs_o, st_sl[0:1, 0:1])
        svo = nc.snap(regs_o, donate=True)
        regs_p = nc.alloc_registers(
            engines=[EngineType.SP, EngineType.Activation])
        nc.regs_load(regs_p, st_sl[0:1, 1:2])
        svp = nc.snap(regs_p, donate=True)

        with ExitStack() as gctx:
            # gram-phase pools: G0 + CT (6MB); AT gets its own pool after
            # the weights/x free up
            gpool = gctx.enter_context(tc.tile_pool(name="gpool", bufs=1))
            g0 = gpool.tile([P, DC, D], F32R, tag="g0")
            ct = gpool.tile([P, DC, DH], F32R, tag="ct")

            with ExitStack() as wctx:
                wpool = wctx.enter_context(tc.tile_pool(name="wpool", bufs=1))
                xpool = wctx.enter_context(tc.tile_pool(name="xpool", bufs=1))
                wkt = wpool.tile([P, DC, D], F32R, tag="wkt")
                wqt = wpool.tile([P, DC, DH], F32R, tag="wqt")
                nc.sync.dma_start(wkt[:], chunked(wk).bitcast(F32R))
                nc.sync.dma_start(wqt[:], chunked(wqh).bitcast(F32R))

                # all 16 x chunks land in one resident tile (no pool-wrap
                # WARs to jitter the stream); both G passes read from it
                xr = xpool.tile([P, XC, D], F32R, tag="xr")
                for c in range(XC):
                    eng = nc.scalar if c % 2 == 0 else nc.gpsimd
                    eng.dma_start(xr[:, c], chunked(xo).bitcast(F32R)[:, c])

                def g_pass(phase, usl):
                    pg = [psum.tile([P, FH], F32, tag="ps",
                                    name=f"g{phase}_{g}") for g in range(DC)]
                    for c in range(XC):
                        for g in range(DC):
                            nc.tensor.matmul(
                                pg[g][:], xr[:, c, g * P:(g + 1) * P],
                                xr[:, c, usl],
                                start=(c == 0), stop=(c == XC - 1))
                    return pg

                # ---- G pass A (u in [0,512)) ----
                pg = g_pass("a", slice(0, FH))
                for g in range(DC):
                    nc.vector.tensor_copy(g0[:, g, 0:FH], pg[g][:])
                nc.sync.dma_start(
                    Gsh[bass.ds(svo, 1), :, 0:FH].rearrange(
                        "s (c p) u -> p (s c) u", p=P),
                    g0[:, :, 0:FH])

                # ---- G pass B (u in [512,1024)) ----
                pg = g_pass("b", slice(FH, D))
                # spill the B-half as two 1MB pieces on two rings: the
                # cross-device writes run ~50-100GB/s per ring, and pieces
                # smaller than ~1MB lose to per-piece latency
                for g in range(DC):
                    nc.vector.tensor_copy(g0[:, g, FH:D], pg[g][:])
                for half in range(2):
                    eng = nc.sync if half == 0 else nc.scalar
                    r0 = half * (DC // 2)
                    eng.dma_start(
                        Gsh[bass.ds(svo, 1), r0 * P:(r0 + DC // 2) * P, FH:D]
                        .rearrange("s (c p) u -> p (s c) u", p=P),
                        g0[:, r0:r0 + DC // 2, FH:D])

                # ---- CT = Wk^T wqh [1024 t, 512 own-d] (hides barrier) ----
                for g in range(DC):
                    pt = psum.tile([P, FH], F32, tag="ps", name=f"ct_{g}")
                    for e in range(DC):
                        nc.tensor.matmul(
                            pt[:], wkt[:, e, g * P:(g + 1) * P], wqt[:, e],
                            start=(e == 0), stop=(e == DC - 1))
                    nc.vector.tensor_copy(ct[:, g], pt[:])

            # ---- pair barrier #1: one token lane per spill piece (RAW dep
            # on each piece's DATA landing in shared DRAM) ----
            tkt = tpool.tile([1, 4], F32, tag="tkt")
            nc.sync.dma_start(tkt[0:1, 0:1], Gsh[bass.ds(svo, 1), 0:1, 0:1]
                              .rearrange("s c n -> c s n").bitcast(F32))
            for half in range(2):
                r0 = half * (DC // 2) * P
                nc.sync.dma_start(
                    tkt[0:1, 1 + half:2 + half],
                    Gsh[bass.ds(svo, 1), r0:r0 + 1, FH:FH + 1]
                    .rearrange("s c n -> c s n").bitcast(F32))
            nc.vector.memset(tkt[0:1, 3:4], 0.0)
            nc.sync.dma_start(tokg[:], tkt[:])
            gbar = nc.gpsimd.collective_compute(
                "AllReduce", mybir.AluOpType.add, replica_groups=PAIRS,
                ins=[tokg], outs=[tokg2])

            with ExitStack() as actx:
                ppool = actx.enter_context(tc.tile_pool(name="ppool", bufs=1))
                at = ppool.tile([P, DC, DH], F32R, tag="at")
                g1 = ppool.tile([P, DC, D], F32R, tag="g1")
                wvt = ppool.tile([P, DC, D], F32R, tag="wvt")
                nc.sync.dma_start(wvt[:], chunked(wvT).bitcast(F32R))

                # ---- AT = (G0 + G1) CT: G0 chunks accumulate before the
                # barrier resolves (covering its latency), G1 chunks chase
                # the peer reads ----
                pa = [psum.tile([P, FH], F32, tag="ps", name=f"at_{u}")
                      for u in range(DC)]
                for c in range(DC):
                    for u in range(DC):
                        nc.tensor.matmul(
                            pa[u][:], g0[:, c, u * P:(u + 1) * P], ct[:, c],
                            start=(c == 0), stop=False)
                for c in range(DC):
                    eng = nc.sync if c % 2 == 0 else nc.scalar
                    ld = eng.dma_start(
                        g1[:, c:c + 1],
                        Gsh[bass.ds(svp, 1), c * P:(c + 1) * P, :].rearrange(
                            "s (c p) u -> p (s c) u", p=P))
                    add_dep_helper(ld.ins, gbar.ins,
                                   reason="peer G after pair barrier")
                for c in range(DC):
                    for u in range(DC):
                        nc.tensor.matmul(
                            pa[u][:], g1[:, c, u * P:(u + 1) * P], ct[:, c],
                            start=False, stop=(c == DC - 1))
                for u in range(DC):
                    nc.vector.tensor_copy(at[:, u], pa[u][:])

                # ---- H own half: [512 own-d, 1024 p] -> hf chunks 0-3.
                # Spilled in two pieces pipelined with the compute so the
                # slow cross-device write overlaps the remaining groups ----
                for dt in range(DH // P):
                    for ph in range(D // FH):
                        pt = psum.tile([P, FH], F32, tag="ps",
                                       name=f"h_{dt}_{ph}")
                        for c in range(DC):
                            nc.tensor.matmul(
                                pt[:], at[:, c, dt * P:(dt + 1) * P],
                                wvt[:, c, ph * FH:(ph + 1) * FH],
                                start=(c == 0), stop=(c == DC - 1))
                        nc.vector.tensor_copy(
                            hf[:, dt, ph * FH:(ph + 1) * FH], pt[:])
                    if dt % 2 == 1:
                        half = dt // 2
                        nc.sync.dma_start(
                            Hsh[bass.ds(svo, 1),
                                half * 2 * P:(half + 1) * 2 * P, :].rearrange(
                                "s (c p) n -> p (s c) n", p=P),
                            hf[:, half * 2:half * 2 + 2])

            # ---- pair barrier #2: token sampled from both spill pieces ----
            tkh = tpool.tile([1, 2], F32, tag="tkh")
            nc.sync.dma_start(tkh[0:1, 0:1], Hsh[bass.ds(svo, 1), 0:1, 0:1]
                              .rearrange("s c n -> c s n").bitcast(F32))
            nc.sync.dma_start(tkh[0:1, 1:2],
                              Hsh[bass.ds(svo, 1), 256:257, 0:1]
                              .rearrange("s c n -> c s n").bitcast(F32))
            nc.sync.dma_start(tokh[:], tkh[:])
            hbar = nc.gpsimd.collective_compute(
                "AllReduce", mybir.AluOpType.add, replica_groups=PAIRS,
                ins=[tokh], outs=[tokh2])
            for c in range(DH // P):
                ld = nc.sync.dma_start(
                    hf[:, DH // P + c:DH // P + c + 1],
                    Hsh[bass.ds(svp, 1), c * P:(c + 1) * P, :].rearrange(
                        "s (c p) n -> p (s c) n", p=P))
                add_dep_helper(ld.ins, hbar.ins,
                               reason="peer H after pair barrier")

        # ---- out = xoTr^T hf (d rotated own-first on host). Two passes:
        # the own-H half-contraction for ALL row tiles first (so the full
        # 35us of own-H work covers barrier #2 + peer-H latency despite the
        # in-order tensor queue), then the peer-H half added in. ----
        with ExitStack() as bctx:
            xtpool = bctx.enter_context(tc.tile_pool(name="xtpool", bufs=1))
            obpool = bctx.enter_context(tc.tile_pool(name="obpool", bufs=1))
            xt = xtpool.tile([P, DC, LH], F32R, tag="xt")
            ob = obpool.tile([P, LH // P, D], F32, tag="ob")
            # all on the scalar ring: gpsimd/sync still have the pending
            # barrier collective + peer reads queued ahead at this point
            for c in range(DC):
                nc.scalar.dma_start(xt[:, c:c + 1],
                                    chunked(xoTr).bitcast(F32R)[:, c:c + 1])

            HC = DC // 2
            for lt in range(LH // P):
                for ph in range(D // FH):
                    pt = psum.tile([P, FH], F32, tag="ps",
                                   name=f"oa_{lt}_{ph}")
                    for c in range(HC):
                        nc.tensor.matmul(
                            pt[:], xt[:, c, lt * P:(lt + 1) * P],
                            hf[:, c, ph * FH:(ph + 1) * FH],
                            start=(c == 0), stop=(c == HC - 1))
                    nc.vector.tensor_copy(
                        ob[:, lt, ph * FH:(ph + 1) * FH], pt[:])
            for lt in range(LH // P):
                for ph in range(D // FH):
                    pt = psum.tile([P, FH], F32, tag="ps",
                                   name=f"ob_{lt}_{ph}")
                    for c in range(HC, DC):
                        nc.tensor.matmul(
                            pt[:], xt[:, c, lt * P:(lt + 1) * P],
                            hf[:, c, ph * FH:(ph + 1) * FH],
                            start=(c == HC), stop=(c == DC - 1))
                    nc.vector.tensor_add(
                        ob[:, lt, ph * FH:(ph + 1) * FH],
                        ob[:, lt, ph * FH:(ph + 1) * FH], pt[:])
                nc.scalar.dma_start(out[lt * P:(lt + 1) * P, :], ob[:, lt])

    nc.compile()
    return nc


_NC_CACHE = {}


def _get_nc():
    if "nc" not in _NC_CACHE:
        _NC_CACHE["nc"] = build_nc()
    return _NC_CACHE["nc"]


def run(inputs, trace=False):
    """Run the kernel on all 8 cores. Returns (full_output, BassKernelResults)."""
    x = np.asarray(inputs["x"], dtype=np.float32)
    Wq = np.asarray(inputs["Wq"], dtype=np.float32)
    Wk = np.asarray(inputs["Wk"], dtype=np.float32)
    Wv = np.asarray(inputs["Wv"], dtype=np.float32)

    inv_sqrt_d = np.float32(1.0 / np.sqrt(D))
    wvT = np.ascontiguousarray(Wv.T)

    in_maps = []
    for c in range(N_CORES):
        b, h = c // 2, c % 2
        xb = x[b, h * LH:(h + 1) * LH, :]          # [2048, 1024]
        xbT = xb.T                                  # [1024, 2048]
        # rotate d-rows of xoT so this core's own d-half comes first
        xoTr = np.concatenate(
            [xbT[h * DH:(h + 1) * DH], xbT[(1 - h) * DH:(2 - h) * DH]],
            axis=0)
        in_maps.append({
            "xo": np.ascontiguousarray(xb),
            "xoTr": np.ascontiguousarray(xoTr),
            "wk": Wk,
            "wqh": np.ascontiguousarray(Wq[:, h * DH:(h + 1) * DH]
                                        * inv_sqrt_d),
            "wvT": wvT,
            "slots": np.array([[h, 1 - h]], dtype=np.uint32),
        })

    nc = _get_nc()
    res = run_bass_kernel_spmd(nc, in_maps, list(range(N_CORES)), trace=trace)

    full = np.empty((B, L, D), dtype=np.float32)
    for c in range(N_CORES):
        b, h = c // 2, c % 2
        full[b, h * LH:(h + 1) * LH, :] = res.results[c]["out"]
    return full, res


def kernel(**inputs):
    full, _ = run(inputs, trace=False)
    return full
